# revision 6
# baseline (speedup 1.0000x reference)
"""3-layer GCN (DGL GraphConv, norm='both') on 8 Trainium2 NeuronCores.

Strategy:
  - Nodes are packed into 80 balanced bins (128 slots each) by in-degree
    (greedy least-loaded), 10 bins per core -> 1280 padded rows/core.
  - Edges live with the owner (bin) of their dst node. segment_sum is done
    as one-hot "scatter matmuls" on the TensorEngine: for each dst block,
    agg[128d, D] += S_kt[128e, 128d].T @ msg_kt[128e, D], where msg rows are
    fetched with dma_gather (SWDGE) and S is a host-built one-hot matrix
    carrying the edge weights norm_src[src]*norm_dst[dst].
  - All feature traffic (h, layer activations, S, W) is bf16: halves both
    the per-edge gather DMA and the AllGather wire bytes vs f32. PSUM
    accumulation stays f32.
  - Layer outputs are exchanged with staged ncfw AllGather slabs
    (SPL = [0,1,3,5,7,9,10] blocks) so the wire time hides under compute.
    Within each dst bin, edges are sorted by the slab of their source row
    and each 4-ktile gather chunk carries a static slab budget; the chunk's
    gather reads a PREFIX of ag_out covering only slabs <= budget, so the
    Tile dependency tracker lets early chunks start before late slabs land.
  - Layer 3 computes y3 = x3 @ W3 locally (padded to 128 wide), AllGathers
    the small y3 in the same staged slabs, and its gathers are issued
    INSIDE the layer-2 block loop as their slabs land, hiding the SWDGE
    descriptor-generation cost; only the last-slab chunks run after L2.
"""
import sys
sys.path.insert(0, '/opt/trn_rl_repo')
import numpy as np
import ml_dtypes

BF16 = ml_dtypes.bfloat16
N_CORES = 8
CH = 4                          # k-tiles per gather chunk (512 rows)


def _ag_splits(nblk):
    """Block-index boundaries of the staged AllGather slabs."""
    if nblk <= 2:
        return [0, nblk]
    fr = [0, round(0.1 * nblk), round(0.3 * nblk), round(0.5 * nblk),
          round(0.7 * nblk), round(0.9 * nblk), nblk]
    return sorted(set(b for b in fr if 0 <= b <= nblk))


def _budgets(nch, nslab):
    """Static max-slab budget per gather chunk (chunk c of every block may
    only reference source rows in slabs <= budget[c])."""
    base = [1, 3, 4]
    out = [min(b, nslab - 1) for b in base[:max(0, min(3, nch - 1))]]
    out += [nslab - 1] * (nch - len(out))
    return out


# ---------------------------------------------------------------- host prep
def _partition_nodes(deg_in, n_nodes, nbins):
    """Greedy balanced-edge binning: nodes (sorted by in-degree desc) go to
    the least-loaded bin with a free slot (capacity 128)."""
    import heapq
    order = np.argsort(-deg_in, kind="stable")
    heap = [(0, b) for b in range(nbins)]
    heapq.heapify(heap)
    bin_of = np.empty(n_nodes, np.int32)
    slot_of = np.empty(n_nodes, np.int32)
    count = np.zeros(nbins, np.int64)
    load = np.zeros(nbins, np.int64)
    for n in order:
        while True:
            l, b = heapq.heappop(heap)
            if count[b] < 128:
                break
            # full bin: drop from heap permanently
        bin_of[n] = b
        slot_of[n] = count[b]
        count[b] += 1
        load[b] += int(deg_in[n])
        heapq.heappush(heap, (l + int(deg_in[n]), b))
    return bin_of, slot_of, load


def _place_chunks(g3, kt_blk, budgets):
    """Assign sorted-by-g3 edge list positions to chunk slots honoring the
    per-chunk slab budgets. Returns slot index per edge or None if the bin
    does not fit."""
    nch = kt_blk // CH
    cap = CH * 128
    n = len(g3)
    slots = np.empty(n, np.int64)
    i = 0
    for c in range(nch):
        take = 0
        while take < cap and i < n and g3[i] <= budgets[c]:
            slots[i] = c * cap + take
            i += 1
            take += 1
    if i < n:
        return None
    return slots


def _prep(h, src, dst, cfg):
    """Build per-core S one-hot tiles, gather indices, and row maps."""
    N, E, NBLK = cfg["N"], cfg["E"], cfg["NBLK"]
    nbins = N_CORES * NBLK
    deg_out = np.bincount(src, minlength=N)
    deg_in = np.bincount(dst, minlength=N)
    norm_src = np.clip(deg_out, 1, None).astype(np.float32) ** np.float32(-0.5)
    norm_dst = np.clip(deg_in, 1, None).astype(np.float32) ** np.float32(-0.5)
    w = (norm_src[src] * norm_dst[dst]).astype(np.float32)

    bin_of, slot_of, load = _partition_nodes(deg_in, N, nbins)

    # deal bins to cores snake-wise by load to balance core totals
    order = np.argsort(-load, kind="stable")
    core_of_bin = np.empty(nbins, np.int32)
    blk_of_bin = np.empty(nbins, np.int32)
    nextblk = [0] * N_CORES
    for i, b in enumerate(order):
        r = i // N_CORES
        c = (i % N_CORES) if r % 2 == 0 else (N_CORES - 1 - (i % N_CORES))
        core_of_bin[b] = c
        blk_of_bin[b] = nextblk[c]
        nextblk[c] += 1

    RPC = NBLK * 128
    row_of_node = (core_of_bin[bin_of] * RPC + blk_of_bin[bin_of] * 128
                   + slot_of).astype(np.int32)
    # gather-id layout after the staged slab AllGathers: slab q holds rows
    # [b_q, e_q) of every core, concatenated core-major at offset 8*b_q
    SPL = _ag_splits(NBLK)
    sp = np.array(SPL) * 128
    _c = row_of_node // RPC
    _r = row_of_node % RPC
    _q = np.searchsorted(sp, _r, side="right") - 1
    gid_of_node = (N_CORES * sp[_q] + _c * (sp[_q + 1] - sp[_q])
                   + _r - sp[_q]).astype(np.int32)
    slab_ends = np.asarray(sp[1:]) * N_CORES    # gid end of each slab
    slab_of_gid = lambda g: np.searchsorted(slab_ends, g, side="right")

    # group edges by dst bin; within a bin order by source slab (g3)
    ebin = bin_of[dst]
    eorder = np.argsort(ebin, kind="stable")
    counts = np.bincount(ebin, minlength=nbins)
    kt_blk = max(cfg["KT_MIN"], int(-(-counts.max() // 128)))
    kt_blk = -(-kt_blk // CH) * CH
    bounds = np.concatenate([[0], np.cumsum(counts)])

    nslab = len(SPL) - 1
    while True:                 # retry with more ktiles if budgets don't fit
        nch = kt_blk // CH
        budgets = _budgets(nch, nslab)
        placements = []
        ok = True
        for b in range(nbins):
            es = eorder[bounds[b]:bounds[b + 1]]
            g3 = slab_of_gid(gid_of_node[src[es]])
            so = np.argsort(g3, kind="stable")
            es = es[so]
            slots = _place_chunks(g3[so], kt_blk, budgets)
            if slots is None:
                ok = False
                break
            placements.append((es, slots))
        if ok:
            break
        kt_blk += CH

    kt_tot = NBLK * kt_blk
    idx1 = np.zeros((N_CORES, kt_tot * 128), np.int16)
    idx23 = np.zeros((N_CORES, kt_tot * 128), np.int16)
    S = np.zeros((N_CORES, 128, kt_tot, 128), np.float32)
    for b in range(nbins):
        es, slots = placements[b]
        c, blk = int(core_of_bin[b]), int(blk_of_bin[b])
        kt = blk * kt_blk + slots // 128
        esl = slots % 128
        gpos = blk * kt_blk * 128 + slots
        idx1[c, gpos] = src[es].astype(np.int16)
        idx23[c, gpos] = gid_of_node[src[es]].astype(np.int16)
        S[c, esl, kt, slot_of[dst[es]]] = w[es]

    def wrap(ix):  # -> [128, kt_tot*8] wrapped for the 8 Q7 cores
        return np.tile(ix.reshape(-1, 16).T, (8, 1)).copy()

    idx1_w = np.stack([wrap(idx1[c]) for c in range(N_CORES)])
    idx23_w = np.stack([wrap(idx23[c]) for c in range(N_CORES)])
    return dict(S=S, idx1=idx1_w, idx23=idx23_w, row_of_node=row_of_node,
                kt_blk=kt_blk, kt_tot=kt_tot)


# ---------------------------------------------------------------- device prog
def _build(cfg, kt_blk, use_bias):
    import concourse.bacc as bacc
    import concourse.mybir as mybir
    import concourse.tile as tile
    from concourse.library_config import mlp

    f32 = mybir.dt.float32
    bf16 = mybir.dt.bfloat16
    i16 = mybir.dt.int16
    RELU = mybir.ActivationFunctionType.Relu
    COPY = mybir.ActivationFunctionType.Copy

    N, D, C, NBLK = cfg["N"], cfg["D"], cfg["C"], cfg["NBLK"]
    CP = 128                    # padded layer-3 width (bf16 gather: 256B rows)
    RPC = NBLK * 128
    NPAD = N_CORES * RPC
    KT = kt_blk
    KT_TOT = NBLK * KT
    NCH = KT // CH
    KD = D // 128               # dense contraction k-tiles
    ND = 512 if D % 512 == 0 else D
    NT = D // ND                # dense n-tiles
    TPW = min(512, D)           # transposes packed per tps tile
    TPG = TPW // 128
    SPL = _ag_splits(NBLK)
    NSLAB = len(SPL) - 1
    sp = [s * 128 for s in SPL]
    BUD = _budgets(NCH, NSLAB)
    PRE = [N_CORES * sp[q + 1] for q in BUD]   # ag_out row prefix per chunk

    nc = bacc.Bacc("TRN2", target_bir_lowering=False, debug=False,
                   num_devices=N_CORES, num_swdge_queues=4,
                   dynamic_dma_scratch_size=32768)

    hx = nc.dram_tensor("hx", [N, D], bf16, kind="ExternalInput")
    sker = nc.dram_tensor("sker", [128, KT_TOT, 128], bf16, kind="ExternalInput")
    idx1_h = nc.dram_tensor("idx1", [128, KT_TOT * 8], i16, kind="ExternalInput")
    idx23_h = nc.dram_tensor("idx23", [128, KT_TOT * 8], i16, kind="ExternalInput")
    w12_h = nc.dram_tensor("w12", [2, 128, KD, D], bf16, kind="ExternalInput")
    w3_h = nc.dram_tensor("w3", [128, KD, CP], bf16, kind="ExternalInput")
    ident_h = nc.dram_tensor("ident", [128, 128], bf16, kind="ExternalInput")
    bias_h = nc.dram_tensor("biases", [1, 2 * D + CP + 128], bf16,
                            kind="ExternalInput")
    out_h = nc.dram_tensor("out", [RPC, C], f32, kind="ExternalOutput")

    ag_in = nc.dram_tensor("ag_in", [RPC, D], bf16, kind="Internal")
    ag_out = nc.dram_tensor("ag_out", [NPAD, D], bf16, kind="Internal",
                            addr_space="Shared")
    ag3_in = nc.dram_tensor("ag3_in", [RPC, CP], bf16, kind="Internal")
    ag3_out = nc.dram_tensor("ag3_out", [NPAD, CP], bf16, kind="Internal",
                             addr_space="Shared")

    with tile.TileContext(nc) as tc:
        nc.gpsimd.load_library(mlp)
        with (
            tc.tile_pool(name="const", bufs=1) as cp,
            tc.tile_pool(name="msg", bufs=5) as mp,
            tc.tile_pool(name="msg3", bufs=NBLK * NCH) as mp3,
            tc.tile_pool(name="work", bufs=2) as wp,
            tc.tile_pool(name="aggps", bufs=2, space="PSUM") as aps,
            tc.tile_pool(name="densps", bufs=2, space="PSUM") as dps,
            tc.tile_pool(name="tpsps", bufs=2, space="PSUM") as tps,
        ):
            idx1_t = cp.tile([128, KT_TOT * 8], i16, tag="idx1")
            nc.sync.dma_start(idx1_t[:], idx1_h[:])
            idx23_t = cp.tile([128, KT_TOT * 8], i16, tag="idx23")
            nc.sync.dma_start(idx23_t[:], idx23_h[:])
            s_blk = []
            for b in range(NBLK):
                sb = cp.tile([128, KT, 128], bf16, tag=f"s{b}")
                nc.sync.dma_start(sb[:], sker[:, b * KT:(b + 1) * KT, :])
                s_blk.append(sb)
            w_t = cp.tile([128, KD, D], bf16, tag="w")
            nc.sync.dma_start(w_t[:], w12_h[0])
            w3_t = cp.tile([128, KD, CP], bf16, tag="w3")
            nc.sync.dma_start(w3_t[:], w3_h[:])
            ident_t = cp.tile([128, 128], bf16, tag="ident")
            nc.sync.dma_start(ident_t[:], ident_h[:])
            if use_bias:
                brow_t = cp.tile([1, 2 * D + CP + 128], bf16, tag="brow")
                nc.sync.dma_start(brow_t[:], bias_h[:])
                ones_t = brow_t[:, 2 * D + CP:2 * D + CP + 128]

            qctr = [0]

            def gather_chunk(b, c, idx_t, width, pool, src_of_chunk, tag="m"):
                """Issue the SWDGE gather for chunk c of dst block b."""
                msg = pool.tile([128, CH, width], bf16, tag=tag)
                col0 = (b * KT + c * CH) * 8
                q = qctr[0] % 4
                qctr[0] += 1
                nc.gpsimd.dma_gather(
                    msg[:], src_of_chunk(c), idx_t[:, col0:col0 + CH * 8],
                    CH * 128, CH * 128, width, queue_num=q)
                return msg

            def spmm_chunk(agg, b, c, msg, width, first, last):
                """Accumulate chunk c's one-hot matmuls into agg."""
                nspl = max(1, width // 512)
                for k in range(CH):
                    kt = c * CH + k
                    for n in range(nspl):
                        w0 = n * (width // nspl)
                        w1 = (n + 1) * (width // nspl)
                        nc.tensor.matmul(
                            agg[:, w0:w1], s_blk[b][:, kt, :],
                            msg[:, k, w0:w1],
                            start=(first and k == 0),
                            stop=(last and k == CH - 1))

            def spmm_block(b, src_of_chunk, idx_t, width, pool, psum_pool,
                           close=True):
                """agg[128, width] for dst block b via gather + one-hot MMs."""
                agg = psum_pool.tile([128, width], f32, tag="aggps")
                for c in range(NCH):
                    msg = gather_chunk(b, c, idx_t, width, pool, src_of_chunk)
                    spmm_chunk(agg, b, c, msg, width,
                               first=(c == 0), last=(c == NCH - 1) and close)
                return agg

            def transpose_to(dst_t, src_sb):
                """dst_t[128, KD, 128] (bf16) = src_sb[128, D] transposed."""
                for g in range(KD // TPG):
                    tp = tps.tile([128, TPW], bf16, tag="tp")
                    for j in range(TPG):
                        col = (g * TPG + j) * 128
                        nc.tensor.transpose(
                            tp[:, j * 128:(j + 1) * 128],
                            src_sb[:, col:col + 128], ident_t[:])
                    nc.vector.tensor_copy(
                        dst_t[:, g * TPG:(g + 1) * TPG, :].rearrange(
                            "p a b -> p (a b)"), tp[:])

            def dense_block(aggT_t, out_sb, bias_off, relu):
                """out_sb[128, D] = act(aggT.T @ W + b)."""
                for n in range(NT):
                    dp = dps.tile([128, ND], f32, tag="dp")
                    for k in range(KD):
                        nc.tensor.matmul(
                            dp[:], aggT_t[:, k, :], w_t[:, k, n * ND:(n + 1) * ND],
                            start=(k == 0), stop=(k == KD - 1 and not use_bias))
                    if use_bias:
                        nc.tensor.matmul(
                            dp[:], ones_t,
                            brow_t[:, bias_off + n * ND:bias_off + (n + 1) * ND],
                            start=False, stop=True)
                    nc.scalar.activation(out_sb[:, n * ND:(n + 1) * ND], dp[:],
                                         RELU if relu else COPY)

            def ag_slab(b, src_h, dst_h, width):
                """Trigger the staged AllGather slab ending at block b+1."""
                if b + 1 in SPL[1:]:
                    r0 = sp[SPL.index(b + 1) - 1]
                    r1 = (b + 1) * 128
                    nc.gpsimd.collective_compute(
                        "AllGather", mybir.AluOpType.bypass,
                        ins=[src_h[r0:r1, :]],
                        outs=[dst_h[N_CORES * r0:N_CORES * r1, :]],
                        replica_groups=[list(range(N_CORES))])

            # layer-3 gather prefetch: issue chunk waves inside the L2 loop
            # once their slab budget's AllGather has safely landed.
            l3_sched = {}
            for blk in range(NBLK):
                for c in range(NCH):
                    if BUD[c] <= 1:
                        at = 5 if blk < NBLK // 2 else 6
                    elif BUD[c] <= 3:
                        at = 8 if blk < NBLK // 2 else 9
                    else:
                        at = NBLK              # after the loop
                    l3_sched.setdefault(min(at, NBLK), []).append((blk, c))
            l3_msg = {}

            def issue_l3(at):
                for blk, c in l3_sched.get(at, []):
                    l3_msg[(blk, c)] = gather_chunk(
                        blk, c, idx23_t, CP, mp3,
                        lambda cc: ag3_out[0:PRE[cc], :],
                        tag="m3")

            # ---------------- layer 1 + 2
            for layer in range(2):
                for b in range(NBLK):
                    if layer == 0:
                        agg = spmm_block(b, lambda c: hx[:], idx1_t, D, mp, aps)
                    else:
                        agg = spmm_block(
                            b, lambda c: ag_out[0:PRE[c], :], idx23_t, D,
                            mp, aps)
                    agg_sb = wp.tile([128, D], bf16, tag="aggsb")
                    nc.scalar.activation(agg_sb[:], agg[:], COPY)
                    aggT_t = wp.tile([128, KD, 128], bf16, tag="aggT")
                    transpose_to(aggT_t, agg_sb)
                    x_sb = wp.tile([128, D], bf16, tag="x")
                    dense_block(aggT_t, x_sb, layer * D, relu=True)
                    if layer == 0:
                        nc.sync.dma_start(ag_in[b * 128:(b + 1) * 128, :], x_sb[:])
                        ag_slab(b, ag_in, ag_out, D)
                    else:
                        # y3 = x3 @ W3 for this block (CP-wide, zero-padded)
                        x3T_t = wp.tile([128, KD, 128], bf16, tag="x3T")
                        transpose_to(x3T_t, x_sb)
                        yp = dps.tile([128, CP], f32, tag="dp")
                        for k in range(KD):
                            nc.tensor.matmul(yp[:], x3T_t[:, k, :], w3_t[:, k, :],
                                             start=(k == 0), stop=(k == KD - 1))
                        y_sb = wp.tile([128, CP], bf16, tag="y")
                        nc.scalar.activation(y_sb[:], yp[:], COPY)
                        nc.sync.dma_start(ag3_in[b * 128:(b + 1) * 128, :], y_sb[:])
                        ag_slab(b, ag3_in, ag3_out, CP)
                        issue_l3(b)
                if layer == 0:
                    nc.sync.dma_start(w_t[:], w12_h[1])
            issue_l3(NBLK)

            # ---------------- layer 3: out = A y3 (+ b3)
            for b in range(NBLK):
                agg3 = aps.tile([128, CP], f32, tag="aggps")
                for c in range(NCH):
                    spmm_chunk(agg3, b, c, l3_msg[(b, c)], CP,
                               first=(c == 0),
                               last=(c == NCH - 1) and not use_bias)
                if use_bias:
                    nc.tensor.matmul(agg3[:], ones_t,
                                     brow_t[:, 2 * D:2 * D + CP],
                                     start=False, stop=True)
                o_sb = wp.tile([128, C], f32, tag="o")
                nc.scalar.activation(o_sb[:], agg3[:, :C], COPY)
                nc.sync.dma_start(out_h[b * 128:(b + 1) * 128, :], o_sb[:])

    nc.compile()
    return nc


_CACHE = {}
_LAST_NC = [None]


def _get_prog(cfg, kt_blk, use_bias):
    key = (cfg["N"], cfg["D"], kt_blk, use_bias)
    if key not in _CACHE:
        _CACHE[key] = _build(cfg, kt_blk, use_bias)
    _LAST_NC[0] = _CACHE[key]
    return _CACHE[key]


# ---------------------------------------------------------------- entry point
CFG_FULL = dict(N=10000, E=160000, D=1024, C=64, NBLK=10, KT_MIN=16)


def _in_maps(ins, cfg=CFG_FULL):
    """Host-side prep: returns (in_maps, prep_dict)."""
    h = np.asarray(ins["h"], np.float32)
    src = np.asarray(ins["src"], np.int32)
    dst = np.asarray(ins["dst"], np.int32)
    D, C = cfg["D"], cfg["C"]
    CP = 128
    KD = D // 128

    pp = _prep(h, src, dst, cfg)
    use_bias = bool(np.any(ins["b1"]) or np.any(ins["b2"]) or np.any(ins["b3"]))
    _get_prog(cfg, pp["kt_blk"], use_bias)

    w12 = np.stack([
        np.asarray(ins["W1"], np.float32).reshape(KD, 128, D).transpose(1, 0, 2),
        np.asarray(ins["W2"], np.float32).reshape(KD, 128, D).transpose(1, 0, 2)])
    w3 = np.zeros((128, KD, CP), np.float32)
    w3[:, :, :C] = np.asarray(ins["W3"], np.float32).reshape(KD, 128, C) \
        .transpose(1, 0, 2)
    biases = np.concatenate([
        np.asarray(ins["b1"], np.float32),
        np.asarray(ins["b2"], np.float32),
        np.asarray(ins["b3"], np.float32), np.zeros(CP - C, np.float32),
        np.ones(128, np.float32)])[None, :]
    ident = np.eye(128, dtype=np.float32)

    in_maps = [
        dict(hx=h.astype(BF16), sker=np.ascontiguousarray(pp["S"][c]).astype(BF16),
             idx1=pp["idx1"][c], idx23=pp["idx23"][c],
             w12=w12.astype(BF16), w3=w3.astype(BF16),
             ident=ident.astype(BF16), biases=biases.astype(BF16))
        for c in range(N_CORES)
    ]
    return in_maps, pp


def kernel(h, src, dst, W1, b1, W2, b2, W3, b3, cfg=CFG_FULL):
    from concourse.bass_utils import run_bass_kernel_spmd

    ins = dict(h=h, src=src, dst=dst, W1=W1, b1=b1, W2=W2, b2=b2,
               W3=W3, b3=b3)
    in_maps, pp = _in_maps(ins, cfg)
    nc = _LAST_NC[0]
    res = run_bass_kernel_spmd(nc, in_maps, core_ids=list(range(N_CORES)))

    N, C = cfg["N"], cfg["C"]
    out = np.zeros((N, C), np.float32)
    rows = pp["row_of_node"]
    allout = np.concatenate([res.results[c]["out"] for c in range(N_CORES)],
                            axis=0)
    out[:, :] = allout[rows]
    return out


# revision 10
# speedup vs baseline: 1.2609x; 1.2609x over previous
"""3-layer GCN (DGL GraphConv, norm='both') on 8 Trainium2 NeuronCores.

Strategy:
  - Nodes are packed into 80 balanced bins (128 slots each) by in-degree
    (greedy least-loaded), 10 bins per core -> 1280 padded rows/core.
  - Edges live with the owner (bin) of their dst node. segment_sum is done
    as one-hot "scatter matmuls" on the TensorEngine: for each dst block,
    agg[128d, D] += S_kt[128e, 128d].T @ msg_kt[128e, D], where msg rows are
    fetched with dma_gather (SWDGE) and S is a host-built one-hot matrix
    carrying the edge weights norm_src[src]*norm_dst[dst].
  - All feature traffic (h, layer activations, S, W) is bf16: halves both
    the per-edge gather DMA and the AllGather wire bytes vs f32. PSUM
    accumulation stays f32.
  - Layer outputs are exchanged with staged ncfw AllGather slabs
    (SPL = [0,1,3,5,7,9,10] blocks). Within each dst bin, edges are sorted
    by the slab of their source row and placed into budgeted chunk ranges
    ([4kt<=slab1, 4kt<=slab3, 8kt<=last]); a chunk's gather reads a PREFIX
    of ag_out covering only its budget slabs, so early chunks start before
    late slabs land.
  - Gathers for block b+1 are EMITTED before block b's AllGather trigger:
    collectives serialize on the Pool engine (each trigger waits for the
    previous collective), so prefetched gathers must sit ahead of that
    wait in the Pool queue or the PE starves (the v3 convoy).
  - Layer 3 computes y3 = x3 @ W3 locally (padded to 128 wide), AllGathers
    the small y3 in the same staged slabs; its first two (budgeted) chunks
    per block are gathered INSIDE the layer-2 block loop as slabs land,
    and only the last-slab chunks run after L2.
"""
import sys
sys.path.insert(0, '/opt/trn_rl_repo')
import numpy as np
import ml_dtypes

BF16 = ml_dtypes.bfloat16
N_CORES = 8


def _ag_splits(nblk):
    """Block-index boundaries of the staged AllGather slabs."""
    if nblk <= 2:
        return [0, nblk]
    fr = [0, round(0.1 * nblk), round(0.3 * nblk), round(0.5 * nblk),
          round(0.7 * nblk), round(0.9 * nblk), nblk]
    return sorted(set(b for b in fr if 0 <= b <= nblk))


def _chunk_plan(kt_blk, nslab):
    """(start_kt, n_kt, budget) triples.

    fine: placement granularity + L2 early blocks + L3 prefetch chunks.
    coarse: L2 later blocks (fewer Pool instructions, coarser deps).
    l1: no budget constraint (hx is always resident).
    """
    b1 = min(1, nslab - 1)
    b3 = min(3, nslab - 1)
    blast = nslab - 1
    if kt_blk <= 8:
        fine = [(0, 4, b1), (4, kt_blk - 4, b3)]
    else:
        fine = [(0, 4, b1), (4, 4, b3), (8, kt_blk - 8, blast)]
    coarse = [(0, 8, b3)] + ([(8, kt_blk - 8, blast)] if kt_blk > 8 else [])
    l1 = [(k, min(8, kt_blk - k), None) for k in range(0, kt_blk, 8)]
    return fine, coarse, l1


# ---------------------------------------------------------------- host prep
def _partition_nodes(deg_in, n_nodes, nbins):
    """Greedy balanced-edge binning: nodes (sorted by in-degree desc) go to
    the least-loaded bin with a free slot (capacity 128)."""
    import heapq
    order = np.argsort(-deg_in, kind="stable")
    heap = [(0, b) for b in range(nbins)]
    heapq.heapify(heap)
    bin_of = np.empty(n_nodes, np.int32)
    slot_of = np.empty(n_nodes, np.int32)
    count = np.zeros(nbins, np.int64)
    load = np.zeros(nbins, np.int64)
    for n in order:
        while True:
            l, b = heapq.heappop(heap)
            if count[b] < 128:
                break
            # full bin: drop from heap permanently
        bin_of[n] = b
        slot_of[n] = count[b]
        count[b] += 1
        load[b] += int(deg_in[n])
        heapq.heappush(heap, (l + int(deg_in[n]), b))
    return bin_of, slot_of, load


def _place_chunks(g3, fine):
    """Assign sorted-by-g3 edges to slots honoring per-chunk slab budgets.
    Returns slot index per edge or None if the bin does not fit."""
    n = len(g3)
    slots = np.empty(n, np.int64)
    i = 0
    for (kt0, nkt, bud) in fine:
        cap = nkt * 128
        base = kt0 * 128
        take = 0
        while take < cap and i < n and g3[i] <= bud:
            slots[i] = base + take
            i += 1
            take += 1
    if i < n:
        return None
    return slots


def _prep(h, src, dst, cfg):
    """Build per-core S one-hot tiles, gather indices, and row maps."""
    N, E, NBLK = cfg["N"], cfg["E"], cfg["NBLK"]
    nbins = N_CORES * NBLK
    deg_out = np.bincount(src, minlength=N)
    deg_in = np.bincount(dst, minlength=N)
    norm_src = np.clip(deg_out, 1, None).astype(np.float32) ** np.float32(-0.5)
    norm_dst = np.clip(deg_in, 1, None).astype(np.float32) ** np.float32(-0.5)
    w = (norm_src[src] * norm_dst[dst]).astype(np.float32)

    bin_of, slot_of, load = _partition_nodes(deg_in, N, nbins)

    # deal bins to cores snake-wise by load to balance core totals
    order = np.argsort(-load, kind="stable")
    core_of_bin = np.empty(nbins, np.int32)
    blk_of_bin = np.empty(nbins, np.int32)
    nextblk = [0] * N_CORES
    for i, b in enumerate(order):
        r = i // N_CORES
        c = (i % N_CORES) if r % 2 == 0 else (N_CORES - 1 - (i % N_CORES))
        core_of_bin[b] = c
        blk_of_bin[b] = nextblk[c]
        nextblk[c] += 1

    RPC = NBLK * 128
    row_of_node = (core_of_bin[bin_of] * RPC + blk_of_bin[bin_of] * 128
                   + slot_of).astype(np.int32)
    # gather-id layout after the staged slab AllGathers: slab q holds rows
    # [b_q, e_q) of every core, concatenated core-major at offset 8*b_q
    SPL = _ag_splits(NBLK)
    sp = np.array(SPL) * 128
    _c = row_of_node // RPC
    _r = row_of_node % RPC
    _q = np.searchsorted(sp, _r, side="right") - 1
    gid_of_node = (N_CORES * sp[_q] + _c * (sp[_q + 1] - sp[_q])
                   + _r - sp[_q]).astype(np.int32)
    slab_ends = np.asarray(sp[1:]) * N_CORES    # gid end of each slab
    slab_of_gid = lambda g: np.searchsorted(slab_ends, g, side="right")

    # group edges by dst bin; within a bin order by source slab (g3)
    ebin = bin_of[dst]
    eorder = np.argsort(ebin, kind="stable")
    counts = np.bincount(ebin, minlength=nbins)
    kt_blk = max(cfg["KT_MIN"], int(-(-counts.max() // 128)))
    kt_blk = -(-kt_blk // 4) * 4
    bounds = np.concatenate([[0], np.cumsum(counts)])

    nslab = len(SPL) - 1
    while True:                 # retry with more ktiles if budgets don't fit
        fine, _, _ = _chunk_plan(kt_blk, nslab)
        placements = []
        ok = True
        for b in range(nbins):
            es = eorder[bounds[b]:bounds[b + 1]]
            g3 = slab_of_gid(gid_of_node[src[es]])
            so = np.argsort(g3, kind="stable")
            es = es[so]
            slots = _place_chunks(g3[so], fine)
            if slots is None:
                ok = False
                break
            placements.append((es, slots))
        if ok:
            break
        kt_blk += 4

    kt_tot = NBLK * kt_blk
    idx1 = np.zeros((N_CORES, kt_tot * 128), np.int16)
    idx23 = np.zeros((N_CORES, kt_tot * 128), np.int16)
    S = np.zeros((N_CORES, 128, kt_tot, 128), np.float32)
    for b in range(nbins):
        es, slots = placements[b]
        c, blk = int(core_of_bin[b]), int(blk_of_bin[b])
        kt = blk * kt_blk + slots // 128
        esl = slots % 128
        gpos = blk * kt_blk * 128 + slots
        idx1[c, gpos] = src[es].astype(np.int16)
        idx23[c, gpos] = gid_of_node[src[es]].astype(np.int16)
        S[c, esl, kt, slot_of[dst[es]]] = w[es]

    def wrap(ix):  # -> [128, kt_tot*8] wrapped for the 8 Q7 cores
        return np.tile(ix.reshape(-1, 16).T, (8, 1)).copy()

    idx1_w = np.stack([wrap(idx1[c]) for c in range(N_CORES)])
    idx23_w = np.stack([wrap(idx23[c]) for c in range(N_CORES)])
    return dict(S=S, idx1=idx1_w, idx23=idx23_w, row_of_node=row_of_node,
                kt_blk=kt_blk, kt_tot=kt_tot)


# ---------------------------------------------------------------- device prog
def _build(cfg, kt_blk, use_bias):
    import concourse.bacc as bacc
    import concourse.mybir as mybir
    import concourse.tile as tile
    from concourse.library_config import mlp

    f32 = mybir.dt.float32
    bf16 = mybir.dt.bfloat16
    i16 = mybir.dt.int16
    RELU = mybir.ActivationFunctionType.Relu
    COPY = mybir.ActivationFunctionType.Copy

    N, D, C, NBLK = cfg["N"], cfg["D"], cfg["C"], cfg["NBLK"]
    CP = 128                    # padded layer-3 width (bf16 gather: 256B rows)
    RPC = NBLK * 128
    NPAD = N_CORES * RPC
    KT = kt_blk
    KT_TOT = NBLK * KT
    KD = D // 128               # dense contraction k-tiles
    ND = 512 if D % 512 == 0 else D
    NT = D // ND                # dense n-tiles
    TPW = min(512, D)           # transposes packed per tps tile
    TPG = TPW // 128
    SPL = _ag_splits(NBLK)
    NSLAB = len(SPL) - 1
    sp = [s * 128 for s in SPL]
    FINE, COARSE, L1CH = _chunk_plan(KT, NSLAB)
    NFINEBLK = min(3, NBLK)     # L2 blocks that use the fine chunk plan

    def prefix(bud):
        return None if bud is None else N_CORES * sp[bud + 1]

    nc = bacc.Bacc("TRN2", target_bir_lowering=False, debug=False,
                   num_devices=N_CORES, num_swdge_queues=4,
                   dynamic_dma_scratch_size=32768)

    hx = nc.dram_tensor("hx", [N, D], bf16, kind="ExternalInput")
    sker = nc.dram_tensor("sker", [128, KT_TOT, 128], bf16, kind="ExternalInput")
    idx1_h = nc.dram_tensor("idx1", [128, KT_TOT * 8], i16, kind="ExternalInput")
    idx23_h = nc.dram_tensor("idx23", [128, KT_TOT * 8], i16, kind="ExternalInput")
    w12_h = nc.dram_tensor("w12", [2, 128, KD, D], bf16, kind="ExternalInput")
    w3_h = nc.dram_tensor("w3", [128, KD, CP], bf16, kind="ExternalInput")
    ident_h = nc.dram_tensor("ident", [128, 128], bf16, kind="ExternalInput")
    bias_h = nc.dram_tensor("biases", [1, 2 * D + CP + 128], bf16,
                            kind="ExternalInput")
    out_h = nc.dram_tensor("out", [RPC, C], f32, kind="ExternalOutput")

    ag_in = nc.dram_tensor("ag_in", [RPC, D], bf16, kind="Internal")
    ag_out = nc.dram_tensor("ag_out", [NPAD, D], bf16, kind="Internal",
                            addr_space="Shared")
    ag3_in = nc.dram_tensor("ag3_in", [RPC, CP], bf16, kind="Internal")
    ag3_out = nc.dram_tensor("ag3_out", [NPAD, CP], bf16, kind="Internal",
                             addr_space="Shared")

    with tile.TileContext(nc) as tc:
        nc.gpsimd.load_library(mlp)
        with (
            tc.tile_pool(name="const", bufs=1) as cp,
            tc.tile_pool(name="m4", bufs=4) as mp4,
            tc.tile_pool(name="m8", bufs=3) as mp8,
            tc.tile_pool(name="m3f", bufs=2 * NBLK) as mp3,
            tc.tile_pool(name="work", bufs=2) as wp,
            tc.tile_pool(name="aggps", bufs=2, space="PSUM") as aps,
            tc.tile_pool(name="densps", bufs=2, space="PSUM") as dps,
            tc.tile_pool(name="tpsps", bufs=2, space="PSUM") as tps,
        ):
            idx1_t = cp.tile([128, KT_TOT * 8], i16, tag="idx1")
            nc.sync.dma_start(idx1_t[:], idx1_h[:])
            idx23_t = cp.tile([128, KT_TOT * 8], i16, tag="idx23")
            nc.sync.dma_start(idx23_t[:], idx23_h[:])
            s_blk = []
            for b in range(NBLK):
                sb = cp.tile([128, KT, 128], bf16, tag=f"s{b}")
                nc.sync.dma_start(sb[:], sker[:, b * KT:(b + 1) * KT, :])
                s_blk.append(sb)
            w_t = cp.tile([128, KD, D], bf16, tag="w")
            nc.sync.dma_start(w_t[:], w12_h[0])
            w3_t = cp.tile([128, KD, CP], bf16, tag="w3")
            nc.sync.dma_start(w3_t[:], w3_h[:])
            ident_t = cp.tile([128, 128], bf16, tag="ident")
            nc.sync.dma_start(ident_t[:], ident_h[:])
            if use_bias:
                brow_t = cp.tile([1, 2 * D + CP + 128], bf16, tag="brow")
                nc.sync.dma_start(brow_t[:], bias_h[:])
                ones_t = brow_t[:, 2 * D + CP:2 * D + CP + 128]

            qctr = [0]

            def gather_chunk(b, kt0, nkt, idx_t, width, src_ap, pool, tag):
                """Issue the SWDGE gather for ktiles [kt0, kt0+nkt) of dst
                block b."""
                msg = pool.tile([128, nkt, width], bf16, tag=tag)
                col0 = (b * KT + kt0) * 8
                q = qctr[0] % 4
                qctr[0] += 1
                nc.gpsimd.dma_gather(
                    msg[:], src_ap, idx_t[:, col0:col0 + nkt * 8],
                    nkt * 128, nkt * 128, width, queue_num=q)
                return msg

            def spmm_chunk(agg, b, kt0, msg, width, first, last):
                """Accumulate a gathered chunk's one-hot matmuls into agg."""
                nkt = msg.shape[1]
                nspl = max(1, width // 512)
                for k in range(nkt):
                    for n in range(nspl):
                        w0 = n * (width // nspl)
                        w1 = (n + 1) * (width // nspl)
                        nc.tensor.matmul(
                            agg[:, w0:w1], s_blk[b][:, kt0 + k, :],
                            msg[:, k, w0:w1],
                            start=(first and k == 0),
                            stop=(last and k == nkt - 1))

            def transpose_to(dst_t, src_sb):
                """dst_t[128, KD, 128] (bf16) = src_sb[128, D] transposed."""
                for g in range(KD // TPG):
                    tp = tps.tile([128, TPW], bf16, tag="tp")
                    for j in range(TPG):
                        col = (g * TPG + j) * 128
                        nc.tensor.transpose(
                            tp[:, j * 128:(j + 1) * 128],
                            src_sb[:, col:col + 128], ident_t[:])
                    nc.vector.tensor_copy(
                        dst_t[:, g * TPG:(g + 1) * TPG, :].rearrange(
                            "p a b -> p (a b)"), tp[:])

            def dense_block(aggT_t, out_sb, bias_off, relu):
                """out_sb[128, D] = act(aggT.T @ W + b)."""
                for n in range(NT):
                    dp = dps.tile([128, ND], f32, tag="dp")
                    for k in range(KD):
                        nc.tensor.matmul(
                            dp[:], aggT_t[:, k, :], w_t[:, k, n * ND:(n + 1) * ND],
                            start=(k == 0), stop=(k == KD - 1 and not use_bias))
                    if use_bias:
                        nc.tensor.matmul(
                            dp[:], ones_t,
                            brow_t[:, bias_off + n * ND:bias_off + (n + 1) * ND],
                            start=False, stop=True)
                    nc.scalar.activation(out_sb[:, n * ND:(n + 1) * ND], dp[:],
                                         RELU if relu else COPY)

            def ag_slab(b, src_h, dst_h):
                """Trigger the staged AllGather slab ending at block b+1."""
                if b + 1 in SPL[1:]:
                    r0 = sp[SPL.index(b + 1) - 1]
                    r1 = (b + 1) * 128
                    nc.gpsimd.collective_compute(
                        "AllGather", mybir.AluOpType.bypass,
                        ins=[src_h[r0:r1, :]],
                        outs=[dst_h[N_CORES * r0:N_CORES * r1, :]],
                        replica_groups=[list(range(N_CORES))])

            def l2_chunks(b):
                return FINE if b < NFINEBLK else COARSE

            def emit_l2_gathers(b, store):
                if b >= NBLK or b in store:
                    return
                store[b] = [
                    gather_chunk(b, kt0, nkt, idx23_t, D,
                                 ag_out[0:prefix(bud), :],
                                 mp4 if nkt <= 4 else mp8,
                                 "m4" if nkt <= 4 else "m8")
                    for (kt0, nkt, bud) in l2_chunks(b)]

            def emit_l1_gathers(b, store):
                if b >= NBLK or b in store:
                    return
                store[b] = [
                    gather_chunk(b, kt0, nkt, idx1_t, D, hx[:], mp8, "m8")
                    for (kt0, nkt, _) in L1CH]

            # layer-3 prefetch: fine chunks with budget q are issued inside
            # the L2 loop once AllGather3 slab q has safely landed
            # (~2 blocks after its trigger at SPL[q+1]-1).
            l3_sched = {}
            for blk in range(NBLK):
                for ci, (kt0, nkt, bud) in enumerate(FINE):
                    if bud >= NSLAB - 1:
                        continue                    # last-slab chunk: post-loop
                    at = min(SPL[bud + 1] + 2 + ((blk * 2) // NBLK), NBLK)
                    l3_sched.setdefault(at, []).append((blk, ci))
            l3_msg = {}

            def issue_l3(at):
                for blk, ci in l3_sched.get(at, []):
                    kt0, nkt, bud = FINE[ci]
                    l3_msg[(blk, ci)] = gather_chunk(
                        blk, kt0, nkt, idx23_t, CP, ag3_out[0:prefix(bud), :],
                        mp3, "m3")

            # ---------------- layer 1 + 2
            for layer in range(2):
                store = {}
                emit = emit_l1_gathers if layer == 0 else emit_l2_gathers
                chunks_of = (lambda b: L1CH) if layer == 0 else l2_chunks
                emit(0, store)
                for b in range(NBLK):
                    emit(b + 1, store)   # prefetch ahead of AG trigger waits
                    agg = aps.tile([128, D], f32, tag="aggps")
                    chs = chunks_of(b)
                    for ci, (kt0, nkt, bud) in enumerate(chs):
                        spmm_chunk(agg, b, kt0, store[b][ci], D,
                                   first=(ci == 0), last=(ci == len(chs) - 1))
                    agg_sb = wp.tile([128, D], bf16, tag="aggsb")
                    nc.scalar.activation(agg_sb[:], agg[:], COPY)
                    aggT_t = wp.tile([128, KD, 128], bf16, tag="aggT")
                    transpose_to(aggT_t, agg_sb)
                    x_sb = wp.tile([128, D], bf16, tag="x")
                    dense_block(aggT_t, x_sb, layer * D, relu=True)
                    if layer == 0:
                        nc.sync.dma_start(ag_in[b * 128:(b + 1) * 128, :], x_sb[:])
                        ag_slab(b, ag_in, ag_out)
                    else:
                        # y3 = x3 @ W3 for this block (CP-wide, zero-padded)
                        x3T_t = wp.tile([128, KD, 128], bf16, tag="x3T")
                        transpose_to(x3T_t, x_sb)
                        yp = dps.tile([128, CP], f32, tag="dp")
                        for k in range(KD):
                            nc.tensor.matmul(yp[:], x3T_t[:, k, :], w3_t[:, k, :],
                                             start=(k == 0), stop=(k == KD - 1))
                        y_sb = wp.tile([128, CP], bf16, tag="y")
                        nc.scalar.activation(y_sb[:], yp[:], COPY)
                        nc.sync.dma_start(ag3_in[b * 128:(b + 1) * 128, :], y_sb[:])
                        ag_slab(b, ag3_in, ag3_out)
                        issue_l3(b)
                if layer == 0:
                    nc.sync.dma_start(w_t[:], w12_h[1])
            issue_l3(NBLK)

            # ---------------- layer 3: out = A y3 (+ b3)
            # last-slab chunks gather now (from the freed m8 ring)
            for b in range(NBLK):
                for ci, (kt0, nkt, bud) in enumerate(FINE):
                    if bud >= NSLAB - 1:
                        l3_msg[(b, ci)] = gather_chunk(
                            b, kt0, nkt, idx23_t, CP, ag3_out[:], mp8, "m8")
            for b in range(NBLK):
                agg3 = aps.tile([128, CP], f32, tag="aggps")
                for ci, (kt0, nkt, bud) in enumerate(FINE):
                    spmm_chunk(agg3, b, kt0, l3_msg[(b, ci)], CP,
                               first=(ci == 0),
                               last=(ci == len(FINE) - 1) and not use_bias)
                if use_bias:
                    nc.tensor.matmul(agg3[:], ones_t,
                                     brow_t[:, 2 * D:2 * D + CP],
                                     start=False, stop=True)
                o_sb = wp.tile([128, C], f32, tag="o")
                nc.scalar.activation(o_sb[:], agg3[:, :C], COPY)
                nc.sync.dma_start(out_h[b * 128:(b + 1) * 128, :], o_sb[:])

    nc.compile()
    return nc


_CACHE = {}
_LAST_NC = [None]


def _get_prog(cfg, kt_blk, use_bias):
    key = (cfg["N"], cfg["D"], kt_blk, use_bias)
    if key not in _CACHE:
        _CACHE[key] = _build(cfg, kt_blk, use_bias)
    _LAST_NC[0] = _CACHE[key]
    return _CACHE[key]


# ---------------------------------------------------------------- entry point
CFG_FULL = dict(N=10000, E=160000, D=1024, C=64, NBLK=10, KT_MIN=16)


def _in_maps(ins, cfg=CFG_FULL):
    """Host-side prep: returns (in_maps, prep_dict)."""
    h = np.asarray(ins["h"], np.float32)
    src = np.asarray(ins["src"], np.int32)
    dst = np.asarray(ins["dst"], np.int32)
    D, C = cfg["D"], cfg["C"]
    CP = 128
    KD = D // 128

    pp = _prep(h, src, dst, cfg)
    use_bias = bool(np.any(ins["b1"]) or np.any(ins["b2"]) or np.any(ins["b3"]))
    _get_prog(cfg, pp["kt_blk"], use_bias)

    w12 = np.stack([
        np.asarray(ins["W1"], np.float32).reshape(KD, 128, D).transpose(1, 0, 2),
        np.asarray(ins["W2"], np.float32).reshape(KD, 128, D).transpose(1, 0, 2)])
    w3 = np.zeros((128, KD, CP), np.float32)
    w3[:, :, :C] = np.asarray(ins["W3"], np.float32).reshape(KD, 128, C) \
        .transpose(1, 0, 2)
    biases = np.concatenate([
        np.asarray(ins["b1"], np.float32),
        np.asarray(ins["b2"], np.float32),
        np.asarray(ins["b3"], np.float32), np.zeros(CP - C, np.float32),
        np.ones(128, np.float32)])[None, :]
    ident = np.eye(128, dtype=np.float32)

    in_maps = [
        dict(hx=h.astype(BF16), sker=np.ascontiguousarray(pp["S"][c]).astype(BF16),
             idx1=pp["idx1"][c], idx23=pp["idx23"][c],
             w12=w12.astype(BF16), w3=w3.astype(BF16),
             ident=ident.astype(BF16), biases=biases.astype(BF16))
        for c in range(N_CORES)
    ]
    return in_maps, pp


def kernel(h, src, dst, W1, b1, W2, b2, W3, b3, cfg=CFG_FULL):
    from concourse.bass_utils import run_bass_kernel_spmd

    ins = dict(h=h, src=src, dst=dst, W1=W1, b1=b1, W2=W2, b2=b2,
               W3=W3, b3=b3)
    in_maps, pp = _in_maps(ins, cfg)
    nc = _LAST_NC[0]
    res = run_bass_kernel_spmd(nc, in_maps, core_ids=list(range(N_CORES)))

    N, C = cfg["N"], cfg["C"]
    out = np.zeros((N, C), np.float32)
    rows = pp["row_of_node"]
    allout = np.concatenate([res.results[c]["out"] for c in range(N_CORES)],
                            axis=0)
    out[:, :] = allout[rows]
    return out


# revision 27
# speedup vs baseline: 1.6080x; 1.2753x over previous
"""3-layer GCN (DGL GraphConv, norm='both') on 8 Trainium2 NeuronCores.

Strategy:
  - Nodes are packed into 80 balanced bins (128 slots each) by in-degree
    (greedy least-loaded), 10 bins per core -> 1280 padded rows/core.
  - Edges live with the owner (bin) of their dst node. segment_sum is done
    as one-hot "scatter matmuls" on the TensorEngine: for each dst block,
    agg[128d, D] += S_kt[128e, 128d].T @ msg_kt[128e, D], where msg rows are
    fetched with dma_gather (SWDGE) and S is a host-built one-hot matrix
    carrying the edge weights norm_src[src]*norm_dst[dst].
  - All feature traffic (h, layer activations, S, W) is bf16: halves both
    the per-edge gather DMA and the AllGather wire bytes vs f32. PSUM
    accumulation stays f32.
  - Layer outputs are exchanged with staged ncfw AllGather slabs
    (SPL = [0,1,3,5,7,9,10] blocks). Within each dst bin, edges are sorted
    by the slab of their source row and placed into budgeted chunk ranges
    ([4kt<=slab1, 4kt<=slab3, 8kt<=last]); a chunk's gather reads a PREFIX
    of ag_out covering only its budget slabs, so early chunks start before
    late slabs land.
  - Gathers for block b+1 are EMITTED before block b's AllGather trigger:
    collectives serialize on the Pool engine (each trigger waits for the
    previous collective), so prefetched gathers must sit ahead of that
    wait in the Pool queue or the PE starves (the v3 convoy).
  - Layer 3 computes y3 = x3 @ W3 locally (padded to 128 wide), AllGathers
    the small y3 in the same staged slabs; its first two (budgeted) chunks
    per block are gathered INSIDE the layer-2 block loop as slabs land,
    and only the last-slab chunks run after L2.
"""
import sys
sys.path.insert(0, '/opt/trn_rl_repo')
import numpy as np
import ml_dtypes

BF16 = ml_dtypes.bfloat16
F8 = ml_dtypes.float8_e4m3fn
N_CORES = 8


def _ag_splits(nblk):
    """Block-index boundaries of the staged AllGather slabs."""
    if nblk <= 2:
        return [0, nblk]
    fr = [0, round(0.1 * nblk), round(0.3 * nblk), round(0.5 * nblk),
          round(0.7 * nblk), round(0.9 * nblk), nblk]
    return sorted(set(b for b in fr if 0 <= b <= nblk))


def _chunk_plan(kt_blk, nslab):
    """(start_kt, n_kt, budget) triples.

    fine: placement granularity + L2 early blocks + L3 prefetch chunks.
    coarse: L2 later blocks (fewer Pool instructions, coarser deps).
    l1: no budget constraint (hx is always resident).
    """
    b1 = min(1, nslab - 1)
    b3 = min(3, nslab - 1)
    blast = nslab - 1
    if kt_blk <= 8:
        fine = [(0, 4, b1), (4, kt_blk - 4, b3)]
    else:
        fine = [(0, 4, b1), (4, 4, b3), (8, kt_blk - 8, blast)]
    coarse = [(0, 8, b3)] + ([(8, kt_blk - 8, blast)] if kt_blk > 8 else [])
    l1 = [(k, min(8, kt_blk - k), None) for k in range(0, kt_blk, 8)]
    return fine, coarse, l1


# ---------------------------------------------------------------- host prep
def _partition_nodes(deg_in, n_nodes, nbins):
    """Greedy balanced-edge binning: nodes (sorted by in-degree desc) go to
    the least-loaded bin with a free slot (capacity 128)."""
    import heapq
    order = np.argsort(-deg_in, kind="stable")
    heap = [(0, b) for b in range(nbins)]
    heapq.heapify(heap)
    bin_of = np.empty(n_nodes, np.int32)
    slot_of = np.empty(n_nodes, np.int32)
    count = np.zeros(nbins, np.int64)
    load = np.zeros(nbins, np.int64)
    for n in order:
        while True:
            l, b = heapq.heappop(heap)
            if count[b] < 128:
                break
            # full bin: drop from heap permanently
        bin_of[n] = b
        slot_of[n] = count[b]
        count[b] += 1
        load[b] += int(deg_in[n])
        heapq.heappush(heap, (l + int(deg_in[n]), b))
    return bin_of, slot_of, load


def _place_chunks(g3, fine):
    """Assign sorted-by-g3 edges to slots honoring per-chunk slab budgets.
    Returns slot index per edge or None if the bin does not fit."""
    n = len(g3)
    slots = np.empty(n, np.int64)
    i = 0
    for (kt0, nkt, bud) in fine:
        cap = nkt * 128
        base = kt0 * 128
        take = 0
        while take < cap and i < n and g3[i] <= bud:
            slots[i] = base + take
            i += 1
            take += 1
    if i < n:
        return None
    return slots


def _prep(h, src, dst, cfg):
    """Build per-core S one-hot tiles, gather indices, and row maps."""
    N, E, NBLK = cfg["N"], cfg["E"], cfg["NBLK"]
    nbins = N_CORES * NBLK
    deg_out = np.bincount(src, minlength=N)
    deg_in = np.bincount(dst, minlength=N)
    norm_src = np.clip(deg_out, 1, None).astype(np.float32) ** np.float32(-0.5)
    norm_dst = np.clip(deg_in, 1, None).astype(np.float32) ** np.float32(-0.5)
    w = (norm_src[src] * norm_dst[dst]).astype(np.float32)

    bin_of, slot_of, load = _partition_nodes(deg_in, N, nbins)

    # deal bins to cores snake-wise by load to balance core totals
    order = np.argsort(-load, kind="stable")
    core_of_bin = np.empty(nbins, np.int32)
    blk_of_bin = np.empty(nbins, np.int32)
    nextblk = [0] * N_CORES
    for i, b in enumerate(order):
        r = i // N_CORES
        c = (i % N_CORES) if r % 2 == 0 else (N_CORES - 1 - (i % N_CORES))
        core_of_bin[b] = c
        blk_of_bin[b] = nextblk[c]
        nextblk[c] += 1

    RPC = NBLK * 128
    row_of_node = (core_of_bin[bin_of] * RPC + blk_of_bin[bin_of] * 128
                   + slot_of).astype(np.int32)
    # per-(slot, block) norm vectors, one pair of columns per block:
    # col b = norm_dst (scales SpMM output rows), col NBLK+b = norm_src
    # (pre-scales stored activations). Norms are factored OUT of the L1/L2
    # one-hot S so it is exactly representable in fp8.
    norms = np.ones((N_CORES, 128, 2 * NBLK), np.float32)
    _cc = core_of_bin[bin_of]
    _bb = blk_of_bin[bin_of]
    norms[_cc, slot_of, _bb] = norm_dst
    norms[_cc, slot_of, NBLK + _bb] = norm_src
    # gather-id layout after the staged slab AllGathers: slab q holds rows
    # [b_q, e_q) of every core, concatenated core-major at offset 8*b_q
    SPL = _ag_splits(NBLK)
    sp = np.array(SPL) * 128
    _c = row_of_node // RPC
    _r = row_of_node % RPC
    _q = np.searchsorted(sp, _r, side="right") - 1
    gid_of_node = (N_CORES * sp[_q] + _c * (sp[_q + 1] - sp[_q])
                   + _r - sp[_q]).astype(np.int32)
    slab_ends = np.asarray(sp[1:]) * N_CORES    # gid end of each slab
    slab_of_gid = lambda g: np.searchsorted(slab_ends, g, side="right")

    # group edges by dst bin; within a bin order by source slab (g3)
    ebin = bin_of[dst]
    eorder = np.argsort(ebin, kind="stable")
    counts = np.bincount(ebin, minlength=nbins)
    kt_blk = max(cfg["KT_MIN"], int(-(-counts.max() // 128)))
    kt_blk = -(-kt_blk // 4) * 4
    bounds = np.concatenate([[0], np.cumsum(counts)])

    nslab = len(SPL) - 1
    while True:                 # retry with more ktiles if budgets don't fit
        fine, _, _ = _chunk_plan(kt_blk, nslab)
        placements = []
        ok = True
        for b in range(nbins):
            es = eorder[bounds[b]:bounds[b + 1]]
            g3 = slab_of_gid(gid_of_node[src[es]])
            so = np.argsort(g3, kind="stable")
            es = es[so]
            slots = _place_chunks(g3[so], fine)
            if slots is None:
                ok = False
                break
            placements.append((es, slots))
        if ok:
            break
        kt_blk += 4

    kt_tot = NBLK * kt_blk
    idx1 = np.zeros((N_CORES, kt_tot * 128), np.int16)
    idx23 = np.zeros((N_CORES, kt_tot * 128), np.int16)
    S = np.zeros((N_CORES, 128, kt_tot, 128), np.float32)
    S01 = np.zeros((N_CORES, 128, kt_tot, 128), np.float32)
    for b in range(nbins):
        es, slots = placements[b]
        c, blk = int(core_of_bin[b]), int(blk_of_bin[b])
        kt = blk * kt_blk + slots // 128
        esl = slots % 128
        gpos = blk * kt_blk * 128 + slots
        idx1[c, gpos] = src[es].astype(np.int16)
        idx23[c, gpos] = gid_of_node[src[es]].astype(np.int16)
        S[c, esl, kt, slot_of[dst[es]]] = w[es]
        S01[c, esl, kt, slot_of[dst[es]]] = 1.0

    def wrap(ix):  # -> [128, kt_tot*8] wrapped for the 8 Q7 cores
        return np.tile(ix.reshape(-1, 16).T, (8, 1)).copy()

    idx1_w = np.stack([wrap(idx1[c]) for c in range(N_CORES)])
    idx23_w = np.stack([wrap(idx23[c]) for c in range(N_CORES)])
    return dict(S=S, S01=S01, idx1=idx1_w, idx23=idx23_w,
                row_of_node=row_of_node, norms=norms,
                kt_blk=kt_blk, kt_tot=kt_tot)


# ---------------------------------------------------------------- device prog
def _build(cfg, kt_blk, use_bias):
    import concourse.bacc as bacc
    import concourse.mybir as mybir
    import concourse.tile as tile
    from concourse.library_config import mlp

    f32 = mybir.dt.float32
    bf16 = mybir.dt.bfloat16
    f8 = mybir.dt.float8e4
    i16 = mybir.dt.int16
    RELU = mybir.ActivationFunctionType.Relu
    COPY = mybir.ActivationFunctionType.Copy

    N, D, C, NBLK = cfg["N"], cfg["D"], cfg["C"], cfg["NBLK"]
    CP = 128                    # padded layer-3 width (bf16 gather: 256B rows)
    RPC = NBLK * 128
    NPAD = N_CORES * RPC
    KT = kt_blk
    KT_TOT = NBLK * KT
    KD = D // 128               # dense contraction k-tiles
    ND = 512 if D % 512 == 0 else D
    NT = D // ND                # dense n-tiles
    TPW = min(512, D)           # transposes packed per tps tile
    TPG = TPW // 128
    SPL = _ag_splits(NBLK)
    NSLAB = len(SPL) - 1
    sp = [s * 128 for s in SPL]
    FINE, COARSE, L1CH = _chunk_plan(KT, NSLAB)
    NFINEBLK = min(3, NBLK)     # L2 blocks that use the fine chunk plan

    def prefix(bud):
        return None if bud is None else N_CORES * sp[bud + 1]

    nc = bacc.Bacc("TRN2", target_bir_lowering=False, debug=False,
                   num_devices=N_CORES, num_swdge_queues=4,
                   dynamic_dma_scratch_size=32768)

    hx = nc.dram_tensor("hx", [N, D], f8, kind="ExternalInput")
    sker = nc.dram_tensor("sker", [128, KT_TOT, 128], bf16, kind="ExternalInput")
    sker8 = nc.dram_tensor("sker8", [128, KT_TOT, 128], f8, kind="ExternalInput")
    idx1_h = nc.dram_tensor("idx1", [128, KT_TOT * 8], i16, kind="ExternalInput")
    idx23_h = nc.dram_tensor("idx23", [128, KT_TOT * 8], i16, kind="ExternalInput")
    w12_h = nc.dram_tensor("w12", [2, 128, KD, D], bf16, kind="ExternalInput")
    w3_h = nc.dram_tensor("w3", [128, KD, CP], bf16, kind="ExternalInput")
    ident_h = nc.dram_tensor("ident", [128, 128], bf16, kind="ExternalInput")
    norms_h = nc.dram_tensor("norms", [128, 2 * NBLK], f32, kind="ExternalInput")
    bias_h = nc.dram_tensor("biases", [1, 2 * D + CP + 128], bf16,
                            kind="ExternalInput")
    out_h = nc.dram_tensor("out", [RPC, C], f32, kind="ExternalOutput")

    ag_in = nc.dram_tensor("ag_in", [RPC, D], f8, kind="Internal")
    ag_out = nc.dram_tensor("ag_out", [NPAD, D], f8, kind="Internal",
                            addr_space="Shared")
    ag3_in = nc.dram_tensor("ag3_in", [RPC, CP], bf16, kind="Internal")
    ag3_out = nc.dram_tensor("ag3_out", [NPAD, CP], bf16, kind="Internal",
                             addr_space="Shared")

    with tile.TileContext(nc) as tc:
        nc.gpsimd.load_library(mlp)
        with (
            tc.tile_pool(name="const", bufs=1) as cp,
            tc.tile_pool(name="m4", bufs=4) as mp4,
            tc.tile_pool(name="m8", bufs=4) as mp8,
            tc.tile_pool(name="m3f", bufs=2 * NBLK) as mp3,
            tc.tile_pool(name="work", bufs=2) as wp,
            tc.tile_pool(name="aggps", bufs=2, space="PSUM") as aps,
            tc.tile_pool(name="densps", bufs=2, space="PSUM") as dps,
            tc.tile_pool(name="tpsps", bufs=2, space="PSUM") as tps,
        ):
            idx1_t = cp.tile([128, KT_TOT * 8], i16, tag="idx1")
            nc.sync.dma_start(idx1_t[:], idx1_h[:])
            idx23_t = cp.tile([128, KT_TOT * 8], i16, tag="idx23")
            nc.sync.dma_start(idx23_t[:], idx23_h[:])
            norms_t = cp.tile([128, 2 * NBLK], f32, tag="norms")
            nc.sync.dma_start(norms_t[:], norms_h[:])
            s8_blk = []
            for b in range(NBLK):
                sb = cp.tile([128, KT, 128], f8, tag=f"s8_{b}")
                nc.sync.dma_start(sb[:], sker8[:, b * KT:(b + 1) * KT, :])
                s8_blk.append(sb)
            s_blk = []
            for b in range(NBLK):
                sb = cp.tile([128, KT, 128], bf16, tag=f"s{b}")
                nc.sync.dma_start(sb[:], sker[:, b * KT:(b + 1) * KT, :])
                s_blk.append(sb)
            w_t = cp.tile([128, KD, D], bf16, tag="w")
            nc.sync.dma_start(w_t[:], w12_h[0])
            w3_t = cp.tile([128, KD, CP], bf16, tag="w3")
            nc.sync.dma_start(w3_t[:], w3_h[:])
            ident_t = cp.tile([128, 128], bf16, tag="ident")
            nc.sync.dma_start(ident_t[:], ident_h[:])
            if use_bias:
                brow_t = cp.tile([1, 2 * D + CP + 128], bf16, tag="brow")
                nc.sync.dma_start(brow_t[:], bias_h[:])
                ones_t = brow_t[:, 2 * D + CP:2 * D + CP + 128]

            qctr = [0]

            def gather_chunk(b, kt0, nkt, idx_t, width, src_ap, pool, tag,
                             dt=f8):
                """Issue the SWDGE gather for ktiles [kt0, kt0+nkt) of dst
                block b."""
                msg = pool.tile([128, nkt, width], dt, tag=tag)
                col0 = (b * KT + kt0) * 8
                q = qctr[0] % 4
                qctr[0] += 1
                nc.gpsimd.dma_gather(
                    msg[:], src_ap, idx_t[:, col0:col0 + nkt * 8],
                    nkt * 128, nkt * 128, width, queue_num=q)
                return msg

            def spmm_chunk(agg, b, kt0, msg, width, first, last, s_tiles):
                """Accumulate a gathered chunk's one-hot matmuls into agg."""
                nkt = msg.shape[1]
                nspl = max(1, width // 512)
                for k in range(nkt):
                    for n in range(nspl):
                        w0 = n * (width // nspl)
                        w1 = (n + 1) * (width // nspl)
                        nc.tensor.matmul(
                            agg[:, w0:w1], s_tiles[b][:, kt0 + k, :],
                            msg[:, k, w0:w1],
                            start=(first and k == 0),
                            stop=(last and k == nkt - 1))

            def transpose_to(dst_t, src_sb):
                """dst_t[128, KD, 128] (bf16) = src_sb[128, D] transposed."""
                for g in range(KD // TPG):
                    tp = tps.tile([128, TPW], bf16, tag="tp")
                    for j in range(TPG):
                        col = (g * TPG + j) * 128
                        nc.tensor.transpose(
                            tp[:, j * 128:(j + 1) * 128],
                            src_sb[:, col:col + 128], ident_t[:])
                    nc.vector.tensor_copy(
                        dst_t[:, g * TPG:(g + 1) * TPG, :].rearrange(
                            "p a b -> p (a b)"), tp[:])

            def dense_block(aggT_t, out_sb, bias_off, relu, out_scale=1.0):
                """out_sb[128, D] = act(aggT.T @ W + b) * out_scale."""
                for n in range(NT):
                    dp = dps.tile([128, ND], f32, tag="dp")
                    for k in range(KD):
                        nc.tensor.matmul(
                            dp[:], aggT_t[:, k, :], w_t[:, k, n * ND:(n + 1) * ND],
                            start=(k == 0), stop=(k == KD - 1 and not use_bias))
                    if use_bias:
                        nc.tensor.matmul(
                            dp[:], ones_t,
                            brow_t[:, bias_off + n * ND:bias_off + (n + 1) * ND],
                            start=False, stop=True)
                    nc.scalar.activation(out_sb[:, n * ND:(n + 1) * ND], dp[:],
                                         RELU if relu else COPY,
                                         scale=out_scale)

            def ag_slab(b, src_h, dst_h):
                """Trigger the staged AllGather slab ending at block b+1."""
                if b + 1 in SPL[1:]:
                    r0 = sp[SPL.index(b + 1) - 1]
                    r1 = (b + 1) * 128
                    nc.gpsimd.collective_compute(
                        "AllGather", mybir.AluOpType.bypass,
                        ins=[src_h[r0:r1, :]],
                        outs=[dst_h[N_CORES * r0:N_CORES * r1, :]],
                        replica_groups=[list(range(N_CORES))])

            def l2_chunks(b):
                return FINE if b < NFINEBLK else COARSE

            def emit_l2_gathers(b, store):
                if b >= NBLK or b in store:
                    return
                store[b] = [
                    gather_chunk(b, kt0, nkt, idx23_t, D,
                                 ag_out[0:prefix(bud), :],
                                 mp4 if nkt <= 4 else mp8,
                                 "m4" if nkt <= 4 else "m8", f8)
                    for (kt0, nkt, bud) in l2_chunks(b)]

            def emit_l1_gathers(b, store):
                if b >= NBLK or b in store:
                    return
                store[b] = [
                    gather_chunk(b, kt0, nkt, idx1_t, D, hx[:], mp8, "m8", f8)
                    for (kt0, nkt, _) in L1CH]

            # layer-3 prefetch: fine chunks with budget q are issued inside
            # the L2 loop once AllGather3 slab q has safely landed
            # (~2 blocks after its trigger at SPL[q+1]-1).
            l3_sched = {}
            for blk in range(NBLK):
                for ci, (kt0, nkt, bud) in enumerate(FINE):
                    if bud >= NSLAB - 1:
                        continue                    # last-slab chunk: post-loop
                    at = min(SPL[bud + 1] + 2 + ((blk * 2) // NBLK), NBLK)
                    l3_sched.setdefault(at, []).append((blk, ci))
            l3_msg = {}

            def issue_l3(at):
                for blk, ci in l3_sched.get(at, []):
                    kt0, nkt, bud = FINE[ci]
                    l3_msg[(blk, ci)] = gather_chunk(
                        blk, kt0, nkt, idx23_t, CP, ag3_out[0:prefix(bud), :],
                        mp3, "m3", bf16)

            # ---------------- layer 1 + 2
            for layer in range(2):
                store = {}
                emit = emit_l1_gathers if layer == 0 else emit_l2_gathers
                chunks_of = (lambda b: L1CH) if layer == 0 else l2_chunks
                emit(0, store)
                for b in range(NBLK):
                    emit(b + 1, store)   # prefetch ahead of AG trigger waits
                    agg = aps.tile([128, D], f32, tag="aggps")
                    chs = chunks_of(b)
                    for ci, (kt0, nkt, bud) in enumerate(chs):
                        spmm_chunk(agg, b, kt0, store[b][ci], D,
                                   first=(ci == 0), last=(ci == len(chs) - 1),
                                   s_tiles=s8_blk)
                    ndst = norms_t[:, b:b + 1]
                    nsrc = norms_t[:, NBLK + b:NBLK + b + 1]
                    agg_sb = wp.tile([128, D], bf16, tag="aggsb")
                    nc.scalar.activation(agg_sb[:], agg[:], COPY, scale=ndst)
                    aggT_t = wp.tile([128, KD, 128], bf16, tag="aggT")
                    transpose_to(aggT_t, agg_sb)
                    if layer == 0:
                        # store x1' = relu(x1) * norm_src as fp8 for L2 gathers
                        x_sb = wp.tile([128, D], f8, tag="x")
                        dense_block(aggT_t, x_sb, 0, relu=True, out_scale=nsrc)
                        nc.sync.dma_start(ag_in[b * 128:(b + 1) * 128, :], x_sb[:])
                        ag_slab(b, ag_in, ag_out)
                    else:
                        x_sb = wp.tile([128, D], bf16, tag="x")
                        dense_block(aggT_t, x_sb, D, relu=True)
                        # y3 = x3 @ W3 for this block (CP-wide, zero-padded);
                        # L3 aggregates with the WEIGHTED bf16 S, so y3 is
                        # stored unscaled.
                        x3T_t = wp.tile([128, KD, 128], bf16, tag="x3T")
                        transpose_to(x3T_t, x_sb)
                        yp = dps.tile([128, CP], f32, tag="dp")
                        for k in range(KD):
                            nc.tensor.matmul(yp[:], x3T_t[:, k, :], w3_t[:, k, :],
                                             start=(k == 0), stop=(k == KD - 1))
                        y_sb = wp.tile([128, CP], bf16, tag="y")
                        nc.scalar.activation(y_sb[:], yp[:], COPY)
                        nc.sync.dma_start(ag3_in[b * 128:(b + 1) * 128, :], y_sb[:])
                        ag_slab(b, ag3_in, ag3_out)
                        issue_l3(b)
                if layer == 0:
                    nc.sync.dma_start(w_t[:], w12_h[1])
            issue_l3(NBLK)

            # ---------------- layer 3: out = A y3 (+ b3)
            # last-slab chunks gather now (from the freed m8 ring)
            for b in range(NBLK):
                for ci, (kt0, nkt, bud) in enumerate(FINE):
                    if bud >= NSLAB - 1:
                        l3_msg[(b, ci)] = gather_chunk(
                            b, kt0, nkt, idx23_t, CP, ag3_out[:], mp8, "m8",
                            bf16)
            for b in range(NBLK):
                agg3 = aps.tile([128, CP], f32, tag="aggps")
                for ci, (kt0, nkt, bud) in enumerate(FINE):
                    spmm_chunk(agg3, b, kt0, l3_msg[(b, ci)], CP,
                               first=(ci == 0),
                               last=(ci == len(FINE) - 1) and not use_bias,
                               s_tiles=s_blk)
                if use_bias:
                    nc.tensor.matmul(agg3[:], ones_t,
                                     brow_t[:, 2 * D:2 * D + CP],
                                     start=False, stop=True)
                o_sb = wp.tile([128, C], f32, tag="o")
                nc.scalar.activation(o_sb[:], agg3[:, :C], COPY)
                nc.sync.dma_start(out_h[b * 128:(b + 1) * 128, :], o_sb[:])

    nc.compile()
    return nc


_CACHE = {}
_LAST_NC = [None]


def _get_prog(cfg, kt_blk, use_bias):
    key = (cfg["N"], cfg["D"], kt_blk, use_bias)
    if key not in _CACHE:
        _CACHE[key] = _build(cfg, kt_blk, use_bias)
    _LAST_NC[0] = _CACHE[key]
    return _CACHE[key]


# ---------------------------------------------------------------- entry point
CFG_FULL = dict(N=10000, E=160000, D=1024, C=64, NBLK=10, KT_MIN=16)


def _in_maps(ins, cfg=CFG_FULL):
    """Host-side prep: returns (in_maps, prep_dict)."""
    h = np.asarray(ins["h"], np.float32)
    src = np.asarray(ins["src"], np.int32)
    dst = np.asarray(ins["dst"], np.int32)
    D, C = cfg["D"], cfg["C"]
    CP = 128
    KD = D // 128

    pp = _prep(h, src, dst, cfg)
    use_bias = bool(np.any(ins["b1"]) or np.any(ins["b2"]) or np.any(ins["b3"]))
    _get_prog(cfg, pp["kt_blk"], use_bias)

    # fp8 stored activations carry the src-side norm; recompute here for hx
    deg_out = np.bincount(src, minlength=cfg["N"])
    nsrc_node = np.clip(deg_out, 1, None).astype(np.float32) ** np.float32(-0.5)
    hq = (h * nsrc_node[:, None]).astype(F8)

    w12 = np.stack([
        np.asarray(ins["W1"], np.float32).reshape(KD, 128, D).transpose(1, 0, 2),
        np.asarray(ins["W2"], np.float32).reshape(KD, 128, D).transpose(1, 0, 2)])
    w3 = np.zeros((128, KD, CP), np.float32)
    w3[:, :, :C] = np.asarray(ins["W3"], np.float32).reshape(KD, 128, C) \
        .transpose(1, 0, 2)
    biases = np.concatenate([
        np.asarray(ins["b1"], np.float32),
        np.asarray(ins["b2"], np.float32),
        np.asarray(ins["b3"], np.float32), np.zeros(CP - C, np.float32),
        np.ones(128, np.float32)])[None, :]
    ident = np.eye(128, dtype=np.float32)

    in_maps = [
        dict(hx=hq, sker=np.ascontiguousarray(pp["S"][c]).astype(BF16),
             sker8=np.ascontiguousarray(pp["S01"][c]).astype(F8),
             idx1=pp["idx1"][c], idx23=pp["idx23"][c],
             w12=w12.astype(BF16), w3=w3.astype(BF16),
             norms=pp["norms"][c],
             ident=ident.astype(BF16), biases=biases.astype(BF16))
        for c in range(N_CORES)
    ]
    return in_maps, pp


def kernel(h, src, dst, W1, b1, W2, b2, W3, b3, cfg=CFG_FULL):
    from concourse.bass_utils import run_bass_kernel_spmd

    ins = dict(h=h, src=src, dst=dst, W1=W1, b1=b1, W2=W2, b2=b2,
               W3=W3, b3=b3)
    in_maps, pp = _in_maps(ins, cfg)
    nc = _LAST_NC[0]
    res = run_bass_kernel_spmd(nc, in_maps, core_ids=list(range(N_CORES)))

    N, C = cfg["N"], cfg["C"]
    out = np.zeros((N, C), np.float32)
    rows = pp["row_of_node"]
    allout = np.concatenate([res.results[c]["out"] for c in range(N_CORES)],
                            axis=0)
    out[:, :] = allout[rows]
    return out
